# revision 1
# baseline (speedup 1.0000x reference)
"""CATAttention Trainium2 kernel.

Math: out[b,i,h,:] = sum_{j<=i} softmax_s(x@W_A^T)[b,i-j,h] * v[b,j,h,:]
i.e. a causal convolution along the sequence with a per-(b,h) data-dependent
kernel z. The [B,H,S,S] "roll" matrix is block-Toeplitz: its 128x128 blocks
depend only on the block lag L = I-J, so only 16 distinct blocks per head are
ever materialized (built in SBUF by a sliding-window DMA from a zero-padded
copy of z in DRAM — the zero pad implements the causal mask for L=0).

Sharding (8 cores): core c -> batch b = c//4, head group g = c%4 (4 heads).
Each core computes z, v = x@W_V^T (its 256 channels), the causal Toeplitz
matmul, and a partial output projection against its 256 columns of W_O.
Host gathers: out[b] = sum of the 4 partials + b_O.

All matmuls run as float32r (fp32 data, single-pass PE mode) which streams at
bf16 rate for moving dims >= 256.
"""

import numpy as np

import concourse.bass as bass
import concourse.mybir as mybir
import concourse.tile as tile
from concourse import masks
from concourse.ap import AP

F32 = mybir.dt.float32
F32R = mybir.dt.float32r
BF16 = mybir.dt.bfloat16
OUT_BF16 = True

B, S, E, H, D = 2, 2048, 1024, 16, 64
SCALING = D ** -0.5
NCORES = 8
HPC = 4            # heads per core
CB = HPC * D       # 256 channels per core
NB = S // 128      # 16 seq blocks
KE = E // 128      # 8 contraction chunks
ZW = 2176          # zpad row width: 128 zeros + 2048 weights


def _split_excess_waits(nc, max_waits=1):
    """The walrus in this container rejects >2 sync waits per instruction.
    Hoist excess waits onto standalone EventSemaphore insts on the same engine."""
    ctr = 0
    for fn in nc.m.functions:
        for bb in fn.blocks:
            out = []
            changed = False
            for inst in list(bb.instructions):
                si = inst.sync_info
                if si is not None and si.on_wait and len(si.on_wait) > max_waits:
                    extra = list(si.on_wait[:-max_waits])
                    keep = list(si.on_wait[-max_waits:])
                    for w in extra:
                        ctr += 1
                        ev = mybir.InstEventSemaphore(
                            name=f"I-waitsplit-{ctr}", ins=[], outs=[]
                        )
                        ev.engine = inst.engine
                        ev.sync_info = mybir.SyncInfo(on_wait=[w], on_update=[])
                        out.append(ev)
                    si.on_wait = keep
                    changed = True
                out.append(inst)
            if changed:
                bb.instructions = out
    return ctr



def _emit_softmax_and_toeplitz(nc, tc, stp, zmax, zraw, nbias, ez, zsum, rz, znR,
                               zpad, zero128, a_sb):
    nc.vector.reduce_max(zmax[:], zraw[:], axis=mybir.AxisListType.X)
    nc.scalar.mul(nbias[:], zmax[:], -SCALING)
    nc.scalar.activation(
        ez[:], zraw[:], mybir.ActivationFunctionType.Exp,
        bias=nbias[:], scale=SCALING, accum_out=zsum[:],
    )
    nc.vector.reciprocal(rz[:], zsum[:])
    # normalize + reverse in one DVE op: znR[h,m] = zn[h, 2047-m]
    nc.vector.tensor_scalar_mul(znR[:], ez[:, ::-1], rz[:])

    # zpad row h holds zn REVERSED (zpad[h,m] = zn_ext[2047-m]) with a
    # zero tail at [2048:2176] (implements the causal mask for L=0).
    nc.sync.dma_start(zpad[:, 0:S], znR[:])
    nc.sync.dma_start(zpad[:, S:ZW], zero128[:].bitcast(F32R))

    # stage_h[j, m] = zpad[h, j + m]  (sliding window, one fat DMA)
    # Toeplitz: A_L^T[j,i] = zn_ext[L*128+i-j] = stage_h[j, 2047-(L*128+i)],
    # so the full reversed stage IS the 16 lag-tiles concatenated: one
    # whole-row reversed DVE copy per head materializes all of them.
    for h in range(HPC):
        stage = stp.tile([128, S], F32R, tag="stage", name=f"stage{h}")
        nc.scalar.dma_start(stage[:], AP(zpad, h * ZW, [[1, 128], [1, S]]))
        nc.vector.tensor_copy(
            a_sb[:, h * S : (h + 1) * S], stage[:, ::-1]
        )


DEFAULT_SPEC = (("conv", 1), ("fin", 1), ("trans", 1), ("v", 1), ("z", 1))


def _build_nc(spec=DEFAULT_SPEC):
    reps = dict(spec)
    diag = any(r > 1 for r in reps.values())
    nc = bass.Bass()
    xT = nc.dram_tensor("xT", [E, S], F32R, kind="ExternalInput")
    # weights arrive host-pre-arranged in the exact SBUF layout (contiguous DMA)
    wat = nc.dram_tensor("wat", [128, KE * HPC], F32R, kind="ExternalInput")
    wvt = nc.dram_tensor("wvt", [128, KE * CB], F32R, kind="ExternalInput")
    wot = nc.dram_tensor("wot", [128, 2 * E], F32R, kind="ExternalInput")
    outp = nc.dram_tensor(
        "outp", [S, E], BF16 if OUT_BF16 else F32, kind="ExternalOutput"
    )
    zpad = nc.dram_tensor("zpad", [HPC, ZW], F32R)

    with tile.TileContext(nc) as tc:
        with (
            tc.tile_pool(name="per", bufs=1) as per,
            tc.tile_pool(name="fs", bufs=2) as fsp,
            tc.tile_pool(name="stp", bufs=(1 if diag else 2)) as stp,
        ):
            ident = per.tile([128, 128], F32, tag="ident")
            masks.make_identity(nc, ident[:])

            xTs = []
            for k in range(KE):
                t = per.tile([128, S], F32R, tag=f"xT{k}", name=f"xTsb{k}")
                xTs.append(t)
            # quartered loads so z/v matmuls can start before the full 8MB lands
            for q in range(4):
                for k in range(KE):
                    nc.sync.dma_start(
                        xTs[k][:, q * 512 : (q + 1) * 512],
                        xT[k * 128 : (k + 1) * 128, q * 512 : (q + 1) * 512],
                    )
            wat_sb = per.tile([128, KE * HPC], F32R, tag="wat")
            nc.sync.dma_start(wat_sb[:], wat[:])
            wvt_sb = per.tile([128, KE * CB], F32R, tag="wvt")
            nc.sync.dma_start(wvt_sb[:], wvt[:])
            wot_sb = per.tile([128, 2 * E], F32R, tag="wot")
            nc.sync.dma_start(wot_sb[:], wot[:])

            zraw = per.tile([HPC, S], F32, tag="zbig")
            ez = per.tile([HPC, S], F32, tag="ez")
            znR = per.tile(
                [HPC, S], F32R, tag=("znR" if diag else "zbig"), name="znR"
            )
            zero128 = per.tile([HPC, 128], F32, tag="zero")
            nc.vector.memset(zero128[:], 0.0)
            zmax = per.tile([HPC, 1], F32, tag="zmax")
            nbias = per.tile([HPC, 1], F32, tag="nbias")
            zsum = per.tile([HPC, 1], F32, tag="zsum")
            rz = per.tile([HPC, 1], F32, tag="rz")

            v_sb = per.tile([128, NB * CB], F32R, tag="v")
            o_sb = per.tile([128, NB * CB], F32, tag="o")
            oTs = [per.tile([128, S], F32R, tag=f"oT{g2}", name=f"oT{g2}") for g2 in range(2)]
            a_sb = per.tile([128, HPC * 16 * 128], F32R, tag="a")

            with (
                tc.tile_pool(name="zp", bufs=2, space="PSUM") as zpool,
                tc.tile_pool(name="vp", bufs=3, space="PSUM") as vpool,
            ):
                # z projection: z^T[h, s] accumulated over 8 e-chunks
                for n in range(4 * reps.get("z", 0)):
                    n = n % 4
                    zp = zpool.tile([HPC, 512], F32, tag="zp")
                    for k in range(KE):
                        nc.tensor.matmul(
                            zp[:],
                            wat_sb[:, k * HPC : (k + 1) * HPC],
                            xTs[k][:, n * 512 : (n + 1) * 512],
                            start=(k == 0),
                            stop=(k == KE - 1),
                        )
                    nc.vector.tensor_copy(zraw[:, n * 512 : (n + 1) * 512], zp[:])

                # softmax over s (free dim): exp(SCALING*(z - max)) / sum
                for _r in range(reps.get("z", 0)):
                    _emit_softmax_and_toeplitz(
                        nc, tc, stp, zmax, zraw, nbias, ez, zsum, rz, znR,
                        zpad, zero128, a_sb)

                # v projection: v[s, c] per seq block, accumulated over e-chunks
                for J in range(NB * reps.get("v", 0)):
                    J = J % NB
                    vp = vpool.tile([128, CB], F32, tag="vp")
                    for k in range(KE):
                        nc.tensor.matmul(
                            vp[:],
                            xTs[k][:, J * 128 : (J + 1) * 128],
                            wvt_sb[:, k * CB : (k + 1) * CB],
                            start=(k == 0),
                            stop=(k == KE - 1),
                        )
                    nc.vector.tensor_copy(v_sb[:, J * CB : (J + 1) * CB], vp[:])

            v3 = v_sb[:].rearrange("p (j c) -> p j c", c=CB)
            o3 = o_sb[:].rearrange("p (i c) -> p i c", c=CB)
            with (
                tc.tile_pool(name="op", bufs=2, space="PSUM") as opool,
                tc.tile_pool(name="tp", bufs=2, space="PSUM") as tpool,
                tc.tile_pool(name="fp", bufs=2, space="PSUM") as fpool,
            ):
                # causal Toeplitz matmul: out_I = sum_L A_L @ V_{I-L}
                # psum cols (I,c); bank0 = out blocks 0..7, bank1 = 8..15
                for h in range(HPC * reps.get("conv", 0)):
                    h = h % HPC
                    op = opool.tile([128, NB * 64], F32, tag="op")
                    for L in range(16):
                        aT = a_sb[
                            :, (h * 16 + L) * 128 : (h * 16 + L + 1) * 128
                        ]
                        n1 = 8 - L
                        if n1 > 0:
                            rhs = v3[:, 0:n1, h * 64 : (h + 1) * 64]
                            nc.tensor.matmul(
                                op[:, L * 64 : 512],
                                aT,
                                rhs,
                                start=(L == 0),
                                stop=(L == 7),
                                skip_group_check=True,
                            )
                        j0 = max(0, 8 - L)
                        rhs = v3[:, j0 : 16 - L, h * 64 : (h + 1) * 64]
                        nc.tensor.matmul(
                            op[:, max(8, L) * 64 : 1024],
                            aT,
                            rhs,
                            start=(L == 0),
                            stop=(L == 15),
                            skip_group_check=True,
                        )
                    nc.vector.tensor_copy(
                        o3[:, :, h * 64 : (h + 1) * 64],
                        op[:].rearrange("p (i c) -> p i c", c=64),
                    )

                # transpose out -> out^T (per 128-channel group) for final proj
                for g2 in range(2 * reps.get("trans", 0)):
                    g2 = g2 % 2
                    for I in range(NB):
                        tp = tpool.tile([128, 128], F32, tag="tp")
                        nc.tensor.transpose(
                            tp[:],
                            o_sb[:, I * CB + g2 * 128 : I * CB + (g2 + 1) * 128],
                            ident[:],
                        )
                        nc.vector.tensor_copy(oTs[g2][:, I * 128 : (I + 1) * 128], tp[:])

                # partial output projection: fin[s, f] = sum_c oT[c, s] wot[c, f]
                for J in range(NB * reps.get("fin", 0)):
                    J = J % NB
                    fs = fsp.tile([128, E], BF16 if OUT_BF16 else F32, tag="fs")
                    for half in range(2):
                        fp = fpool.tile([128, 512], F32, tag="fp")
                        for cc in range(2):
                            nc.tensor.matmul(
                                fp[:],
                                oTs[cc][:, J * 128 : (J + 1) * 128],
                                wot_sb[
                                    :, cc * E + half * 512 : cc * E + (half + 1) * 512
                                ],
                                start=(cc == 0),
                                stop=(cc == 1),
                            )
                        nc.vector.tensor_copy(
                            fs[:, half * 512 : (half + 1) * 512], fp[:]
                        )
                    nc.sync.dma_start(outp[J * 128 : (J + 1) * 128, :], fs[:])

    _split_excess_waits(nc)
    return nc


class _Runner:
    """Builds the Bass module once and keeps the jitted shard_map executable."""

    def __init__(self, spec=DEFAULT_SPEC):
        import jax
        from jax.sharding import Mesh, PartitionSpec

        try:
            from jax.experimental.shard_map import shard_map
        except ImportError:
            from jax.shard_map import shard_map

        from concourse import bass2jax

        bass2jax.install_neuronx_cc_hook()
        self.jax = jax
        nc = _build_nc(spec)
        self.nc = nc

        partition_name = (
            nc.partition_id_tensor.name if nc.partition_id_tensor else None
        )
        in_names, out_names, out_avals, zero_outs = [], [], [], []
        for alloc in nc.m.functions[0].allocations:
            if not isinstance(alloc, mybir.MemoryLocationSet):
                continue
            name = alloc.memorylocations[0].name
            if alloc.kind == "ExternalInput":
                if name != partition_name:
                    in_names.append(name)
            elif alloc.kind == "ExternalOutput":
                shape = tuple(alloc.tensor_shape)
                dtype = mybir.dt.np(alloc.dtype)
                out_names.append(name)
                out_avals.append(jax.core.ShapedArray(shape, dtype))
                zero_outs.append(np.zeros(shape, dtype))
        self.in_names = in_names
        self.out_names = out_names
        self.out_shapes = [tuple(a.shape) for a in out_avals]
        self.zero_outs = zero_outs
        n_params = len(in_names)
        n_outs = len(out_names)
        all_in_names = list(in_names) + list(out_names)
        if partition_name is not None:
            all_in_names.append(partition_name)

        def _body(*args):
            operands = list(args)
            if partition_name is not None:
                operands.append(bass2jax.partition_id_tensor())
            outs = bass2jax._bass_exec_p.bind(
                *operands,
                out_avals=tuple(out_avals),
                in_names=tuple(all_in_names),
                out_names=tuple(out_names),
                lowering_input_output_aliases=(),
                sim_require_finite=True,
                sim_require_nnan=True,
                nc=nc,
            )
            return tuple(outs)

        devices = jax.devices()[:NCORES]
        assert len(devices) == NCORES, f"need {NCORES} cores, got {len(devices)}"
        self.mesh = Mesh(np.asarray(devices), ("core",))
        in_specs = (PartitionSpec("core"),) * (n_params + n_outs)
        out_specs = (PartitionSpec("core"),) * n_outs
        donate = tuple(range(n_params, n_params + n_outs))
        self.sharded = jax.jit(
            shard_map(
                _body,
                mesh=self.mesh,
                in_specs=in_specs,
                out_specs=out_specs,
                check_rep=False,
            ),
            donate_argnums=donate,
            keep_unused=True,
        )
        # Non-donating variant for benchmarking: one zeros set can be reused
        # across dispatches (kernel writes every output element).
        self.sharded_nodonate = jax.jit(
            shard_map(
                _body,
                mesh=self.mesh,
                in_specs=in_specs,
                out_specs=out_specs,
                check_rep=False,
            ),
            keep_unused=True,
        )

    def concat_inputs(self, in_maps):
        return [
            np.concatenate([np.asarray(in_maps[c][nm]) for c in range(NCORES)], axis=0)
            for nm in self.in_names
        ]

    def fresh_zeros(self):
        return [
            np.zeros((NCORES * z.shape[0], *z.shape[1:]), z.dtype)
            for z in self.zero_outs
        ]

    def run_concat(self, concat_in, zeros):
        out_arrs = self.sharded(*concat_in, *zeros)
        return out_arrs

    def run(self, in_maps):
        out_arrs = self.run_concat(self.concat_inputs(in_maps), self.fresh_zeros())
        res = []
        for c in range(NCORES):
            res.append(
                {
                    nm: np.asarray(out_arrs[i]).reshape(
                        NCORES, *self.out_shapes[i]
                    )[c]
                    for i, nm in enumerate(self.out_names)
                }
            )
        return res


_RUNNERS = {}


def _get_runner(spec=DEFAULT_SPEC):
    spec = tuple(sorted(dict(spec).items()))
    if spec not in _RUNNERS:
        _RUNNERS[spec] = _Runner(spec)
    return _RUNNERS[spec]


def _shard_inputs(x, W_A, W_V, W_O):
    x = np.asarray(x, dtype=np.float32)
    W_A = np.asarray(W_A, dtype=np.float32)
    W_V = np.asarray(W_V, dtype=np.float32)
    W_O = np.asarray(W_O, dtype=np.float32)
    xTs = [np.ascontiguousarray(x[b].T) for b in range(B)]

    def sb_layout(wT, nk):
        # [nk*128, c] -> [128, nk*c]: partition p holds chunk-k cols at k*c
        c = wT.shape[1]
        return np.ascontiguousarray(
            wT.reshape(nk, 128, c).transpose(1, 0, 2).reshape(128, nk * c)
        )

    in_maps = []
    for c in range(NCORES):
        b, g = divmod(c, NCORES // B)
        r0, r1 = g * CB, (g + 1) * CB
        in_maps.append(
            {
                "xT": xTs[b],
                "wat": sb_layout(W_A[g * HPC : (g + 1) * HPC, :].T, KE),
                "wvt": sb_layout(W_V[r0:r1, :].T, KE),
                "wot": sb_layout(W_O[:, r0:r1].T, 2),
            }
        )
    return in_maps


def kernel(x, W_A, W_V, W_O, b_O):
    runner = _get_runner()
    in_maps = _shard_inputs(x, W_A, W_V, W_O)
    res = runner.run(in_maps)
    b_O = np.asarray(b_O, dtype=np.float32)
    out = np.empty((B, S, E), np.float32)
    gpb = NCORES // B
    for b in range(B):
        acc = res[b * gpb]["outp"].astype(np.float32)
        for g in range(1, gpb):
            acc = acc + res[b * gpb + g]["outp"].astype(np.float32)
        out[b] = acc + b_O
    return out


def _marginal_once(runner, dev_in, zset, k_small=4, k_big=64):
    import time

    def run_k(k):
        t0 = time.perf_counter()
        outs = None
        for _ in range(k):
            outs = runner.sharded_nodonate(*dev_in, *zset)
        for a in outs:
            a.block_until_ready()
        return time.perf_counter() - t0

    t_small = run_k(k_small)
    t_big = run_k(k_big)
    return (t_big - t_small) / (k_big - k_small) * 1e6


def measure_exec_ns(x, W_A, W_V, W_O, b_O, amp=17, pairs=7):
    """Per-execution device time: interleaved paired marginals of the normal
    kernel vs an `amp`-times-repeated body (drift-cancelling)."""
    import jax
    from jax.sharding import NamedSharding, PartitionSpec

    in_maps = _shard_inputs(x, W_A, W_V, W_O)
    setups = {}
    for factor in (1, amp):
        spec = tuple((p, factor) for p in ("z", "v", "conv", "fin", "trans"))
        runner = _get_runner(spec)
        sh = NamedSharding(runner.mesh, PartitionSpec("core"))
        dev_in = [jax.device_put(a, sh) for a in runner.concat_inputs(in_maps)]
        zset = [jax.device_put(z, sh) for z in runner.fresh_zeros()]
        for a in zset:
            a.block_until_ready()
        # warm
        _marginal_once(runner, dev_in, zset, 1, 2)
        setups[factor] = (runner, dev_in, zset)
    diffs = []
    m1s, mAs = [], []
    for _ in range(pairs):
        m1 = _marginal_once(*setups[1])
        mA = _marginal_once(*setups[amp])
        m1s.append(m1)
        mAs.append(mA)
        diffs.append((mA - m1) / (amp - 1))
    diffs.sort()
    med = diffs[len(diffs) // 2]
    return {
        "m1_us": [round(v) for v in m1s],
        f"m{amp}_us": [round(v) for v in mAs],
        "diffs_us": [round(v, 1) for v in sorted(diffs)],
        "per_exec_ns": int(med * 1e3),
    }



# revision 23
# speedup vs baseline: 1.9448x; 1.9448x over previous
"""CATAttention Trainium2 kernel (v2 — bf16, DMA/pipeline restructure).

Math: out[b,i,h,:] = sum_{j<=i} softmax_s(x@W_A^T)[b,i-j,h] * v[b,j,h,:]
i.e. a causal convolution along the sequence with a per-(b,h) data-dependent
kernel z. The [B,H,S,S] "roll" matrix is block-Toeplitz: its 128x128 blocks
depend only on the block lag L = I-J, so only 16 distinct blocks per head are
ever materialized. They are built UNNORMALIZED (from ez = exp(logits)) by a
single negative-stride sliding-window DMA per head from a zero-padded copy of
ez in DRAM; the 1/sum(ez) softmax normalizer is folded into the PSUM->SBUF
copy of the conv output (per-head scalar).

Sharding (8 cores): core c -> batch b = c//4, head group g = c%4 (4 heads).
Each core computes z, v = x@W_V^T (its 256 channels), the causal Toeplitz
matmul, and a partial output projection against its 256 columns of W_O.
Host gathers: out[b] = sum of the 4 partials + b_O.

All data bf16 (f32 PSUM accumulation). Input DMAs are 4 fat quarter loads
with the weights interleaved so the z/v matmuls start at ~3us, not ~26us.
"""

import numpy as np

import concourse.bass as bass
import concourse.mybir as mybir
import concourse.tile as tile
from concourse import masks
from concourse.ap import AP

F32 = mybir.dt.float32
BF16 = mybir.dt.bfloat16
NPBF16 = mybir.dt.np(BF16)
OUT_BF16 = True

B, S, E, H, D = 2, 2048, 1024, 16, 64
SCALING = D ** -0.5
NCORES = 8
HPC = 4            # heads per core
CB = HPC * D       # 256 channels per core
NB = S // 128      # 16 seq blocks
KE = E // 128      # 8 contraction chunks
ZW = S + 128       # zpad row: 128 zeros + 2048 ez values


def _split_excess_waits(nc, max_waits=1):
    """The walrus in this container rejects >2 sync waits per instruction.
    Hoist excess waits onto standalone EventSemaphore insts on the same engine."""
    ctr = 0
    for fn in nc.m.functions:
        for bb in fn.blocks:
            out = []
            changed = False
            for inst in list(bb.instructions):
                si = inst.sync_info
                if si is not None and si.on_wait and len(si.on_wait) > max_waits:
                    extra = list(si.on_wait[:-max_waits])
                    keep = list(si.on_wait[-max_waits:])
                    for w in extra:
                        ctr += 1
                        ev = mybir.InstEventSemaphore(
                            name=f"I-waitsplit-{ctr}", ins=[], outs=[]
                        )
                        ev.engine = inst.engine
                        ev.sync_info = mybir.SyncInfo(on_wait=[w], on_update=[])
                        out.append(ev)
                    si.on_wait = keep
                    changed = True
                out.append(inst)
            if changed:
                bb.instructions = out
    return ctr


DEFAULT_SPEC = (("conv", 1), ("fin", 1), ("trans", 1), ("v", 1), ("z", 1))


def _build_nc(spec=DEFAULT_SPEC):
    reps = dict(spec)
    nc = bass.Bass()
    xT = nc.dram_tensor("xT", [E, S], BF16, kind="ExternalInput")
    # weights arrive host-pre-arranged in the exact SBUF layout (contiguous DMA)
    wat = nc.dram_tensor("wat", [128, KE * HPC], BF16, kind="ExternalInput")
    wvt = nc.dram_tensor("wvt", [128, KE * CB], BF16, kind="ExternalInput")
    wot = nc.dram_tensor("wot", [128, 2 * E], BF16, kind="ExternalInput")
    outp = nc.dram_tensor(
        "outp", [S, E], BF16 if OUT_BF16 else F32, kind="ExternalOutput"
    )
    zpad = nc.dram_tensor("zpad", [HPC, ZW], BF16)
    rzd = nc.dram_tensor("rzd", [1, HPC], F32)

    with tile.TileContext(nc) as tc:
        with (
            tc.tile_pool(name="per", bufs=1) as per,
            tc.tile_pool(name="fs", bufs=4) as fsp,
            tc.tile_pool(name="stp", bufs=2) as stp,
        ):
            ident = per.tile([128, 128], BF16, tag="ident")
            masks.make_identity(nc, ident[:])

            # --- input loads: weights interleaved with 4 fat x-quarter DMAs
            wat_sb = per.tile([128, KE * HPC], BF16, tag="wat")
            wvt_sb = per.tile([128, KE * CB], BF16, tag="wvt")
            wot_sb = per.tile([128, 2 * E], BF16, tag="wot")
            xT_sb = per.tile([128, KE * S], BF16, tag="xT")
            x3 = xT_sb[:].rearrange("p (k c) -> p k c", k=KE)

            def load_xq(n, k0=0, k1=KE):
                # one DMA: k-chunks [k0,k1) columns [n*512, (n+1)*512)
                nc.sync.dma_start(
                    x3[:, k0:k1, n * 512 : (n + 1) * 512],
                    AP(
                        xT,
                        k0 * 128 * S + n * 512,
                        [[S, 128], [128 * S, k1 - k0], [1, 512]],
                    ),
                )

            nc.sync.dma_start(wat_sb[:], wat[:])
            load_xq(0, 0, 4)
            load_xq(0, 4, KE)
            nc.sync.dma_start(wvt_sb[:], wvt[:])
            for n in range(1, 4):
                load_xq(n)
            nc.sync.dma_start(wot_sb[:], wot[:])

            # zero tail of zpad (t=0, Pool queue); exp act-table preload (Act)
            zero128 = per.tile([HPC, 128], BF16, tag="zero")
            nc.vector.memset(zero128[:], 0.0)
            nc.gpsimd.dma_start(
                AP(zpad, S, [[ZW, HPC], [1, 128]]), zero128[:]
            )
            dumm = per.tile([1, 1], F32, tag="dumm")
            nc.scalar.activation(
                dumm[:], zero128[0:1, 0:1], mybir.ActivationFunctionType.Exp
            )

            ez = per.tile([HPC, S], BF16, tag="ez")
            ezR = per.tile([HPC, S], BF16, tag="ezR")
            zsum2 = per.tile([HPC, 2], F32, tag="zsum2")
            zsum = per.tile([HPC, 1], F32, tag="zsum")
            rz128 = per.tile([128, HPC], F32, tag="rz128")
            rzr = per.tile([128, HPC], F32, tag="rzr")

            v_sb = per.tile([128, NB * CB], BF16, tag="v")
            o_sb = per.tile([128, NB * CB], BF16, tag="o")
            oTs = [
                per.tile([128, S], BF16, tag=f"oT{g2}", name=f"oT{g2}")
                for g2 in range(2)
            ]
            a_sb = per.tile([128, HPC * S], BF16, tag="a")

            with (
                tc.tile_pool(name="zp", bufs=1, space="PSUM") as zpool,
                tc.tile_pool(name="vp", bufs=3, space="PSUM") as vpool,
            ):
                zp = zpool.tile([HPC, S], F32, tag="zp")

                def emit_z_matmuls(n):
                    for k in range(KE):
                        nc.tensor.matmul(
                            zp[:, n * 512 : (n + 1) * 512],
                            wat_sb[:, k * HPC : (k + 1) * HPC],
                            x3[:, k, n * 512 : (n + 1) * 512],
                            start=(k == 0),
                            stop=(k == KE - 1),
                            skip_group_check=True,
                        )

                def emit_v_block(J):
                    vp = vpool.tile([128, CB], F32, tag="vp")
                    for k in range(KE):
                        nc.tensor.matmul(
                            vp[:],
                            x3[:, k, J * 128 : (J + 1) * 128],
                            wvt_sb[:, k * CB : (k + 1) * CB],
                            start=(k == 0),
                            stop=(k == KE - 1),
                        )
                    nc.vector.tensor_copy(v_sb[:, J * CB : (J + 1) * CB], vp[:])

                stages = []

                def emit_exp_half(hf):
                    # exp reads the z PSUM directly; sum rides along in accum.
                    # zpad rows hold ez REVERSED: zpad[h,m] = ez[h, S-1-m],
                    # zero tail at [S, ZW) (written at t=0); data writes go
                    # on the (idle) gpsimd SWDGE queue.
                    c0, c1 = hf * 1024, (hf + 1) * 1024
                    nc.scalar.activation(
                        ez[:, c0:c1], zp[:, c0:c1],
                        mybir.ActivationFunctionType.Exp,
                        scale=SCALING, accum_out=zsum2[:, hf : hf + 1],
                    )
                    r0 = S - c1  # reversed position of this half
                    nc.vector.tensor_copy(
                        ezR[:, r0 : r0 + 1024], ez[:, c0:c1][:, ::-1]
                    )
                    nc.gpsimd.dma_start(
                        AP(zpad, r0, [[ZW, HPC], [1, 1024]]),
                        ezR[:, r0 : r0 + 1024],
                    )

                def emit_stage():
                    nc.vector.reduce_sum(
                        zsum[:], zsum2[:], axis=mybir.AxisListType.X
                    )
                    # zsum broadcast to all partitions via DRAM bounce
                    # ([4,1] partitions -> 4 consecutive floats -> [128,4])
                    nc.sync.dma_start(
                        AP(rzd, 0, [[1, HPC], [1, 1]]), zsum[:]
                    )
                    nc.sync.dma_start(
                        rz128[:], AP(rzd, 0, [[0, 128], [1, HPC]])
                    )
                    # stage_h[j, m] = zpad[h, j + m]  (sliding window; zpad
                    # holds ez reversed, zero tail) -> the whole-row reversed
                    # DVE copy below materializes all 16 lag blocks at once:
                    # a_sb[j, h*S + q] = stage_h[j, S-1-q] = ez_ext[q - j]
                    stages.clear()
                    for h in range(HPC):
                        st = stp.tile([128, S], BF16, tag="stage", name=f"st{h}")
                        nc.scalar.dma_start(
                            st[:], AP(zpad, h * ZW, [[1, 128], [1, S]])
                        )
                        stages.append(st)

                def emit_rev(h):
                    nc.vector.tensor_copy(
                        a_sb[:, h * S : (h + 1) * S], stages[h][:, ::-1]
                    )

                # PE order: each z quarter as soon as its x quarter lands,
                # v blocks filling the slack
                def emit_v_range(j0, j1):
                    for _r in range(max(1, reps.get("v", 0))):
                        for J in range(j0, j1):
                            emit_v_block(J)

                for _r in range(reps.get("z", 0)):
                    emit_z_matmuls(0)
                emit_v_range(0, 3)
                for _r in range(reps.get("z", 0)):
                    emit_z_matmuls(1)
                    emit_exp_half(0)
                emit_v_range(3, 5)
                for _r in range(reps.get("z", 0)):
                    emit_z_matmuls(2)
                emit_v_range(5, 7)
                for _r in range(reps.get("z", 0)):
                    emit_z_matmuls(3)
                    emit_exp_half(1)
                    emit_stage()
                emit_v_range(7, 14)
                for _r in range(reps.get("z", 0)):
                    for h in range(HPC):
                        emit_rev(h)
                emit_v_range(14, NB)

            v3 = v_sb[:].rearrange("p (j c) -> p j c", c=CB)
            o3 = o_sb[:].rearrange("p (i c) -> p i c", c=CB)
            with (
                tc.tile_pool(name="op", bufs=3, space="PSUM") as opool,
                tc.tile_pool(name="tp", bufs=2, space="PSUM") as tpool,
                tc.tile_pool(name="fp", bufs=3, space="PSUM") as fpool,
            ):
                # softmax normalizer, needed first by the conv copy-out
                nc.vector.reciprocal(rzr[:], rz128[:])

                # causal Toeplitz matmul, halved: half 0 = out blocks I 0..7,
                # half 1 = I 8..15. Per (half, head): one PSUM bank.
                def emit_conv(h, half):
                    op = opool.tile([128, 512], F32, tag="op")
                    if half == 0:
                        for L in range(8):
                            aT = a_sb[:, (h * 16 + L) * 128 : (h * 16 + L + 1) * 128]
                            nc.tensor.matmul(
                                op[:, L * 64 : 512],
                                aT,
                                v3[:, 0 : 8 - L, h * 64 : (h + 1) * 64],
                                start=(L == 0),
                                stop=(L == 7),
                                skip_group_check=True,
                            )
                        dst = o3[:, 0:8, h * 64 : (h + 1) * 64]
                    else:
                        for L in range(16):
                            aT = a_sb[:, (h * 16 + L) * 128 : (h * 16 + L + 1) * 128]
                            j0 = max(0, 8 - L)
                            nc.tensor.matmul(
                                op[:, (max(8, L) - 8) * 64 : 512],
                                aT,
                                v3[:, j0 : 16 - L, h * 64 : (h + 1) * 64],
                                start=(L == 0),
                                stop=(L == 15),
                                skip_group_check=True,
                            )
                        dst = o3[:, 8:NB, h * 64 : (h + 1) * 64]
                    # fold softmax 1/sum into the copy-out
                    nc.vector.tensor_scalar_mul(
                        dst,
                        op[:].rearrange("p (i c) -> p i c", c=64),
                        rzr[:, h : h + 1],
                    )

                # transpose o -> oT for out blocks I in [i0, i0+4), group g2
                def emit_trans(g2, igrp):
                    tp = tpool.tile([128, 512], BF16, tag="tp")
                    for i in range(4):
                        I = igrp * 4 + i
                        nc.tensor.transpose(
                            tp[:, i * 128 : (i + 1) * 128],
                            o_sb[:, I * CB + g2 * 128 : I * CB + (g2 + 1) * 128],
                            ident[:],
                        )
                    nc.vector.tensor_copy(
                        oTs[g2][:, igrp * 512 : (igrp + 1) * 512], tp[:]
                    )

                # partial output projection for seq block J
                def emit_fin(J):
                    fs = fsp.tile([128, E], BF16 if OUT_BF16 else F32, tag="fs")
                    for half in range(2):
                        fp = fpool.tile([128, 512], F32, tag="fp")
                        for cc in range(2):
                            nc.tensor.matmul(
                                fp[:],
                                oTs[cc][:, J * 128 : (J + 1) * 128],
                                wot_sb[
                                    :, cc * E + half * 512 : cc * E + (half + 1) * 512
                                ],
                                start=(cc == 0),
                                stop=(cc == 1),
                            )
                        if half == 0:
                            nc.vector.tensor_copy(
                                fs[:, half * 512 : (half + 1) * 512], fp[:]
                            )
                        else:
                            nc.scalar.copy(
                                fs[:, half * 512 : (half + 1) * 512], fp[:]
                            )
                    nc.sync.dma_start(outp[J * 128 : (J + 1) * 128, :], fs[:])

                # PE pipeline: convA -> convB(h0) -> transA -> convB(h1) ->
                # finA -> convB(h2,h3) -> transB -> finB
                for _r in range(max(1, reps.get("conv", 0))):
                    for h in range(HPC):
                        emit_conv(h, 0)
                    emit_conv(0, 1)
                for _r in range(max(1, reps.get("trans", 0))):
                    emit_trans(0, 0)
                    emit_trans(1, 0)
                    emit_trans(0, 1)
                    emit_trans(1, 1)
                for _r in range(max(1, reps.get("conv", 0))):
                    emit_conv(1, 1)
                for _r in range(max(1, reps.get("fin", 0))):
                    for J in range(0, 8):
                        emit_fin(J)
                for _r in range(max(1, reps.get("conv", 0))):
                    emit_conv(2, 1)
                    emit_conv(3, 1)
                for _r in range(max(1, reps.get("trans", 0))):
                    emit_trans(0, 2)
                    emit_trans(1, 2)
                    emit_trans(0, 3)
                    emit_trans(1, 3)
                for _r in range(max(1, reps.get("fin", 0))):
                    for J in range(8, NB):
                        emit_fin(J)

    _split_excess_waits(nc)
    return nc


class _Runner:
    """Builds the Bass module once and keeps the jitted shard_map executable."""

    def __init__(self, spec=DEFAULT_SPEC):
        import jax
        from jax.sharding import Mesh, PartitionSpec

        try:
            from jax.experimental.shard_map import shard_map
        except ImportError:
            from jax.shard_map import shard_map

        from concourse import bass2jax

        bass2jax.install_neuronx_cc_hook()
        self.jax = jax
        nc = _build_nc(spec)
        self.nc = nc

        partition_name = (
            nc.partition_id_tensor.name if nc.partition_id_tensor else None
        )
        in_names, out_names, out_avals, zero_outs = [], [], [], []
        for alloc in nc.m.functions[0].allocations:
            if not isinstance(alloc, mybir.MemoryLocationSet):
                continue
            name = alloc.memorylocations[0].name
            if alloc.kind == "ExternalInput":
                if name != partition_name:
                    in_names.append(name)
            elif alloc.kind == "ExternalOutput":
                shape = tuple(alloc.tensor_shape)
                dtype = mybir.dt.np(alloc.dtype)
                out_names.append(name)
                out_avals.append(jax.core.ShapedArray(shape, dtype))
                zero_outs.append(np.zeros(shape, dtype))
        self.in_names = in_names
        self.out_names = out_names
        self.out_shapes = [tuple(a.shape) for a in out_avals]
        self.zero_outs = zero_outs
        n_params = len(in_names)
        n_outs = len(out_names)
        all_in_names = list(in_names) + list(out_names)
        if partition_name is not None:
            all_in_names.append(partition_name)

        def _body(*args):
            operands = list(args)
            if partition_name is not None:
                operands.append(bass2jax.partition_id_tensor())
            outs = bass2jax._bass_exec_p.bind(
                *operands,
                out_avals=tuple(out_avals),
                in_names=tuple(all_in_names),
                out_names=tuple(out_names),
                lowering_input_output_aliases=(),
                sim_require_finite=True,
                sim_require_nnan=True,
                nc=nc,
            )
            return tuple(outs)

        devices = jax.devices()[:NCORES]
        assert len(devices) == NCORES, f"need {NCORES} cores, got {len(devices)}"
        self.mesh = Mesh(np.asarray(devices), ("core",))
        in_specs = (PartitionSpec("core"),) * (n_params + n_outs)
        out_specs = (PartitionSpec("core"),) * n_outs
        donate = tuple(range(n_params, n_params + n_outs))
        self.sharded = jax.jit(
            shard_map(
                _body,
                mesh=self.mesh,
                in_specs=in_specs,
                out_specs=out_specs,
                check_rep=False,
            ),
            donate_argnums=donate,
            keep_unused=True,
        )
        # Non-donating variant for benchmarking: one zeros set can be reused
        # across dispatches (kernel writes every output element).
        self.sharded_nodonate = jax.jit(
            shard_map(
                _body,
                mesh=self.mesh,
                in_specs=in_specs,
                out_specs=out_specs,
                check_rep=False,
            ),
            keep_unused=True,
        )

    def concat_inputs(self, in_maps):
        return [
            np.concatenate([np.asarray(in_maps[c][nm]) for c in range(NCORES)], axis=0)
            for nm in self.in_names
        ]

    def fresh_zeros(self):
        return [
            np.zeros((NCORES * z.shape[0], *z.shape[1:]), z.dtype)
            for z in self.zero_outs
        ]

    def run_concat(self, concat_in, zeros):
        out_arrs = self.sharded(*concat_in, *zeros)
        return out_arrs

    def run(self, in_maps):
        out_arrs = self.run_concat(self.concat_inputs(in_maps), self.fresh_zeros())
        res = []
        for c in range(NCORES):
            res.append(
                {
                    nm: np.asarray(out_arrs[i]).reshape(
                        NCORES, *self.out_shapes[i]
                    )[c]
                    for i, nm in enumerate(self.out_names)
                }
            )
        return res


_RUNNERS = {}


def _get_runner(spec=DEFAULT_SPEC):
    spec = tuple(sorted(dict(spec).items()))
    if spec not in _RUNNERS:
        _RUNNERS[spec] = _Runner(spec)
    return _RUNNERS[spec]


def _shard_inputs(x, W_A, W_V, W_O):
    x = np.asarray(x, dtype=np.float32)
    W_A = np.asarray(W_A, dtype=np.float32)
    W_V = np.asarray(W_V, dtype=np.float32)
    W_O = np.asarray(W_O, dtype=np.float32)
    xTs = [np.ascontiguousarray(x[b].T).astype(NPBF16) for b in range(B)]

    def sb_layout(wT, nk):
        # [nk*128, c] -> [128, nk*c]: partition p holds chunk-k cols at k*c
        c = wT.shape[1]
        return np.ascontiguousarray(
            wT.reshape(nk, 128, c).transpose(1, 0, 2).reshape(128, nk * c)
        ).astype(NPBF16)

    in_maps = []
    for c in range(NCORES):
        b, g = divmod(c, NCORES // B)
        r0, r1 = g * CB, (g + 1) * CB
        in_maps.append(
            {
                "xT": xTs[b],
                "wat": sb_layout(W_A[g * HPC : (g + 1) * HPC, :].T, KE),
                "wvt": sb_layout(W_V[r0:r1, :].T, KE),
                "wot": sb_layout(W_O[:, r0:r1].T, 2),
            }
        )
    return in_maps


def kernel(x, W_A, W_V, W_O, b_O):
    runner = _get_runner()
    in_maps = _shard_inputs(x, W_A, W_V, W_O)
    res = runner.run(in_maps)
    b_O = np.asarray(b_O, dtype=np.float32)
    out = np.empty((B, S, E), np.float32)
    gpb = NCORES // B
    for b in range(B):
        acc = res[b * gpb]["outp"].astype(np.float32)
        for g in range(1, gpb):
            acc = acc + res[b * gpb + g]["outp"].astype(np.float32)
        out[b] = acc + b_O
    return out


def _marginal_once(runner, dev_in, zset, k_small=4, k_big=64):
    import time

    def run_k(k):
        t0 = time.perf_counter()
        outs = None
        for _ in range(k):
            outs = runner.sharded_nodonate(*dev_in, *zset)
        for a in outs:
            a.block_until_ready()
        return time.perf_counter() - t0

    t_small = run_k(k_small)
    t_big = run_k(k_big)
    return (t_big - t_small) / (k_big - k_small) * 1e6


def measure_exec_ns(x, W_A, W_V, W_O, b_O, amp=17, pairs=7):
    """Per-execution device time: interleaved paired marginals of the normal
    kernel vs an `amp`-times-repeated body (drift-cancelling)."""
    import jax
    from jax.sharding import NamedSharding, PartitionSpec

    in_maps = _shard_inputs(x, W_A, W_V, W_O)
    setups = {}
    for factor in (1, amp):
        spec = tuple((p, factor) for p in ("z", "v", "conv", "fin", "trans"))
        runner = _get_runner(spec)
        sh = NamedSharding(runner.mesh, PartitionSpec("core"))
        dev_in = [jax.device_put(a, sh) for a in runner.concat_inputs(in_maps)]
        zset = [jax.device_put(z, sh) for z in runner.fresh_zeros()]
        for a in zset:
            a.block_until_ready()
        # warm
        _marginal_once(runner, dev_in, zset, 1, 2)
        setups[factor] = (runner, dev_in, zset)
    diffs = []
    m1s, mAs = [], []
    for _ in range(pairs):
        m1 = _marginal_once(*setups[1])
        mA = _marginal_once(*setups[amp])
        m1s.append(m1)
        mAs.append(mA)
        diffs.append((mA - m1) / (amp - 1))
    diffs.sort()
    med = diffs[len(diffs) // 2]
    return {
        "m1_us": [round(v) for v in m1s],
        f"m{amp}_us": [round(v) for v in mAs],
        "diffs_us": [round(v, 1) for v in sorted(diffs)],
        "per_exec_ns": int(med * 1e3),
    }


# revision 27
# speedup vs baseline: 3.1163x; 1.6024x over previous
"""CATAttention Trainium2 kernel (v2 — bf16, DMA/pipeline restructure).

Math: out[b,i,h,:] = sum_{j<=i} softmax_s(x@W_A^T)[b,i-j,h] * v[b,j,h,:]
i.e. a causal convolution along the sequence with a per-(b,h) data-dependent
kernel z. The [B,H,S,S] "roll" matrix is block-Toeplitz: its 128x128 blocks
depend only on the block lag L = I-J, so only 16 distinct blocks per head are
ever materialized. They are built UNNORMALIZED (from ez = exp(logits)) by a
single negative-stride sliding-window DMA per head from a zero-padded copy of
ez in DRAM; the 1/sum(ez) softmax normalizer is folded into the PSUM->SBUF
copy of the conv output (per-head scalar).

Sharding (8 cores): core c -> batch b = c//4, head group g = c%4 (4 heads).
Each core computes z, v = x@W_V^T (its 256 channels), the causal Toeplitz
matmul, and a partial output projection against its 256 columns of W_O.
Host gathers: out[b] = sum of the 4 partials + b_O.

All data bf16 (f32 PSUM accumulation). Input DMAs are 4 fat quarter loads
with the weights interleaved so the z/v matmuls start at ~3us, not ~26us.
"""

import numpy as np

import concourse.bass as bass
import concourse.mybir as mybir
import concourse.tile as tile
from concourse import masks
from concourse.ap import AP

F32 = mybir.dt.float32
BF16 = mybir.dt.bfloat16
NPBF16 = mybir.dt.np(BF16)
OUT_BF16 = True

B, S, E, H, D = 2, 2048, 1024, 16, 64
SCALING = D ** -0.5
NCORES = 8
HPC = 4            # heads per core
CB = HPC * D       # 256 channels per core
NB = S // 128      # 16 seq blocks
KE = E // 128      # 8 contraction chunks
ZW = S + 128       # zpad row: 128 zeros + 2048 ez values


def _split_excess_waits(nc, max_waits=1):
    """The walrus in this container rejects >2 sync waits per instruction.
    Hoist excess waits onto standalone EventSemaphore insts on the same engine."""
    ctr = 0
    for fn in nc.m.functions:
        for bb in fn.blocks:
            out = []
            changed = False
            for inst in list(bb.instructions):
                si = inst.sync_info
                if si is not None and si.on_wait and len(si.on_wait) > max_waits:
                    extra = list(si.on_wait[:-max_waits])
                    keep = list(si.on_wait[-max_waits:])
                    for w in extra:
                        ctr += 1
                        ev = mybir.InstEventSemaphore(
                            name=f"I-waitsplit-{ctr}", ins=[], outs=[]
                        )
                        ev.engine = inst.engine
                        ev.sync_info = mybir.SyncInfo(on_wait=[w], on_update=[])
                        out.append(ev)
                    si.on_wait = keep
                    changed = True
                out.append(inst)
            if changed:
                bb.instructions = out
    return ctr


DEFAULT_SPEC = (("conv", 1), ("fin", 1), ("trans", 1), ("v", 1), ("z", 1))


def _build_nc(spec=DEFAULT_SPEC):
    reps = dict(spec)
    nc = bass.Bass()
    xT = nc.dram_tensor("xT", [E, S], BF16, kind="ExternalInput")
    # weights arrive host-pre-arranged in the exact SBUF layout (contiguous DMA)
    wat = nc.dram_tensor("wat", [128, KE * HPC], BF16, kind="ExternalInput")
    wvt = nc.dram_tensor("wvt", [128, KE * CB], BF16, kind="ExternalInput")
    wot = nc.dram_tensor("wot", [128, 2 * E], BF16, kind="ExternalInput")
    outp = nc.dram_tensor(
        "outp", [S, E], BF16 if OUT_BF16 else F32, kind="ExternalOutput"
    )
    zpad = nc.dram_tensor("zpad", [HPC, ZW], BF16)
    rzd = nc.dram_tensor("rzd", [1, HPC], F32)

    with tile.TileContext(nc) as tc:
        with (
            tc.tile_pool(name="per", bufs=1) as per,
            tc.tile_pool(name="fs", bufs=4) as fsp,
            tc.tile_pool(name="stp", bufs=2) as stp,
        ):
            ident = per.tile([128, 128], BF16, tag="ident")
            masks.make_identity(nc, ident[:])

            # --- input loads: weights interleaved with 4 fat x-quarter DMAs
            wat_sb = per.tile([128, KE * HPC], BF16, tag="wat")
            wvt_sb = per.tile([128, KE * CB], BF16, tag="wvt")
            wot_sb = per.tile([128, 2 * E], BF16, tag="wot")
            xT_sb = per.tile([128, KE * S], BF16, tag="xT")
            x3 = xT_sb[:].rearrange("p (k c) -> p k c", k=KE)

            def load_xq(n, k0=0, k1=KE):
                # one DMA: k-chunks [k0,k1) columns [n*512, (n+1)*512)
                nc.sync.dma_start(
                    x3[:, k0:k1, n * 512 : (n + 1) * 512],
                    AP(
                        xT,
                        k0 * 128 * S + n * 512,
                        [[S, 128], [128 * S, k1 - k0], [1, 512]],
                    ),
                )

            nc.sync.dma_start(wat_sb[:], wat[:])
            load_xq(0, 0, 4)
            load_xq(0, 4, KE)
            nc.sync.dma_start(wvt_sb[:], wvt[:])
            for n in range(1, 4):
                load_xq(n)
            nc.sync.dma_start(wot_sb[:], wot[:])

            # zero tail of zpad (t=0, Pool queue); exp act-table preload (Act)
            zero128 = per.tile([HPC, 128], BF16, tag="zero")
            nc.vector.memset(zero128[:], 0.0)
            nc.gpsimd.dma_start(
                AP(zpad, S, [[ZW, HPC], [1, 128]]), zero128[:]
            )
            dumm = per.tile([1, 1], F32, tag="dumm")
            nc.scalar.activation(
                dumm[:], zero128[0:1, 0:1], mybir.ActivationFunctionType.Exp
            )

            ez = per.tile([HPC, S], BF16, tag="ez")
            ezR = per.tile([HPC, S], BF16, tag="ezR")
            zsum2 = per.tile([HPC, 2], F32, tag="zsum2")
            zsum = per.tile([HPC, 1], F32, tag="zsum")
            rz128 = per.tile([128, HPC], F32, tag="rz128")
            rzr = per.tile([128, HPC], F32, tag="rzr")

            v_sb = per.tile([128, NB * CB], BF16, tag="v")
            o_sb = per.tile([128, NB * CB], BF16, tag="o")
            oTs = [
                per.tile([128, S], BF16, tag=f"oT{g2}", name=f"oT{g2}")
                for g2 in range(2)
            ]
            a_sb = per.tile([128, HPC * S], BF16, tag="a")

            with (
                tc.tile_pool(name="zp", bufs=1, space="PSUM") as zpool,
                tc.tile_pool(name="vp", bufs=3, space="PSUM") as vpool,
            ):
                zp = zpool.tile([HPC, S], F32, tag="zp")

                def emit_z_matmuls(n):
                    for k in range(KE):
                        nc.tensor.matmul(
                            zp[:, n * 512 : (n + 1) * 512],
                            wat_sb[:, k * HPC : (k + 1) * HPC],
                            x3[:, k, n * 512 : (n + 1) * 512],
                            start=(k == 0),
                            stop=(k == KE - 1),
                            skip_group_check=True,
                        )

                def emit_v_block(J):
                    vp = vpool.tile([128, CB], F32, tag="vp")
                    for k in range(KE):
                        nc.tensor.matmul(
                            vp[:],
                            x3[:, k, J * 128 : (J + 1) * 128],
                            wvt_sb[:, k * CB : (k + 1) * CB],
                            start=(k == 0),
                            stop=(k == KE - 1),
                        )
                    nc.vector.tensor_copy(v_sb[:, J * CB : (J + 1) * CB], vp[:])

                stages = []

                def emit_exp_half(hf):
                    # exp reads the z PSUM directly; sum rides along in accum.
                    # zpad rows hold ez REVERSED: zpad[h,m] = ez[h, S-1-m],
                    # zero tail at [S, ZW) (written at t=0); data writes go
                    # on the (idle) gpsimd SWDGE queue.
                    c0, c1 = hf * 1024, (hf + 1) * 1024
                    nc.scalar.activation(
                        ez[:, c0:c1], zp[:, c0:c1],
                        mybir.ActivationFunctionType.Exp,
                        scale=SCALING, accum_out=zsum2[:, hf : hf + 1],
                    )
                    r0 = S - c1  # reversed position of this half
                    nc.vector.tensor_copy(
                        ezR[:, r0 : r0 + 1024], ez[:, c0:c1][:, ::-1]
                    )
                    nc.gpsimd.dma_start(
                        AP(zpad, r0, [[ZW, HPC], [1, 1024]]),
                        ezR[:, r0 : r0 + 1024],
                    )

                def emit_stage():
                    nc.vector.reduce_sum(
                        zsum[:], zsum2[:], axis=mybir.AxisListType.X
                    )
                    # zsum broadcast to all partitions via DRAM bounce
                    # ([4,1] partitions -> 4 consecutive floats -> [128,4])
                    nc.sync.dma_start(
                        AP(rzd, 0, [[1, HPC], [1, 1]]), zsum[:]
                    )
                    nc.sync.dma_start(
                        rz128[:], AP(rzd, 0, [[0, 128], [1, HPC]])
                    )
                    # stage_h[j, m] = zpad[h, j + m]  (sliding window; zpad
                    # holds ez reversed, zero tail) -> the whole-row reversed
                    # DVE copy below materializes all 16 lag blocks at once:
                    # a_sb[j, h*S + q] = stage_h[j, S-1-q] = ez_ext[q - j]
                    stages.clear()
                    for h in range(HPC):
                        st = stp.tile([128, S], BF16, tag="stage", name=f"st{h}")
                        nc.scalar.dma_start(
                            st[:], AP(zpad, h * ZW, [[1, 128], [1, S]])
                        )
                        stages.append(st)

                def emit_rev(h):
                    nc.vector.tensor_copy(
                        a_sb[:, h * S : (h + 1) * S], stages[h][:, ::-1]
                    )

                # PE order: each z quarter as soon as its x quarter lands,
                # v blocks filling the slack
                def emit_v_range(j0, j1):
                    for _r in range(max(1, reps.get("v", 0))):
                        for J in range(j0, j1):
                            emit_v_block(J)

                for _r in range(reps.get("z", 0)):
                    emit_z_matmuls(0)
                emit_v_range(0, 3)
                for _r in range(reps.get("z", 0)):
                    emit_z_matmuls(1)
                    emit_exp_half(0)
                emit_v_range(3, 5)
                for _r in range(reps.get("z", 0)):
                    emit_z_matmuls(2)
                emit_v_range(5, 7)
                for _r in range(reps.get("z", 0)):
                    emit_z_matmuls(3)
                    emit_exp_half(1)
                    emit_stage()
                emit_v_range(7, 14)
                for _r in range(reps.get("z", 0)):
                    for h in range(HPC):
                        emit_rev(h)
                emit_v_range(14, NB)

            v3 = v_sb[:].rearrange("p (j c) -> p j c", c=CB)
            o3 = o_sb[:].rearrange("p (i c) -> p i c", c=CB)
            with (
                tc.tile_pool(name="op", bufs=3, space="PSUM") as opool,
                tc.tile_pool(name="tp", bufs=2, space="PSUM") as tpool,
                tc.tile_pool(name="fp", bufs=3, space="PSUM") as fpool,
            ):
                # softmax normalizer, needed first by the conv copy-out
                nc.vector.reciprocal(rzr[:], rz128[:])

                # causal Toeplitz matmul, halved: half 0 = out blocks I 0..7,
                # half 1 = I 8..15. Per (half, head): one PSUM bank.
                def emit_conv(h, half):
                    op = opool.tile([128, 512], F32, tag="op")
                    if half == 0:
                        for L in range(8):
                            aT = a_sb[:, (h * 16 + L) * 128 : (h * 16 + L + 1) * 128]
                            nc.tensor.matmul(
                                op[:, L * 64 : 512],
                                aT,
                                v3[:, 0 : 8 - L, h * 64 : (h + 1) * 64],
                                start=(L == 0),
                                stop=(L == 7),
                                skip_group_check=True,
                            )
                        dst = o3[:, 0:8, h * 64 : (h + 1) * 64]
                    else:
                        for L in range(16):
                            aT = a_sb[:, (h * 16 + L) * 128 : (h * 16 + L + 1) * 128]
                            j0 = max(0, 8 - L)
                            nc.tensor.matmul(
                                op[:, (max(8, L) - 8) * 64 : 512],
                                aT,
                                v3[:, j0 : 16 - L, h * 64 : (h + 1) * 64],
                                start=(L == 0),
                                stop=(L == 15),
                                skip_group_check=True,
                            )
                        dst = o3[:, 8:NB, h * 64 : (h + 1) * 64]
                    # fold softmax 1/sum into the copy-out; spread the copies
                    # over DVE (heads 0,1) and gpsimd (heads 2,3)
                    eng = nc.vector if h < 2 else nc.gpsimd
                    eng.tensor_scalar_mul(
                        dst,
                        op[:].rearrange("p (i c) -> p i c", c=64),
                        rzr[:, h : h + 1],
                    )

                # transpose o -> oT for out blocks I in [i0, i0+4), group g2
                def emit_trans(g2, igrp):
                    tp = tpool.tile([128, 512], BF16, tag="tp")
                    for i in range(4):
                        I = igrp * 4 + i
                        nc.tensor.transpose(
                            tp[:, i * 128 : (i + 1) * 128],
                            o_sb[:, I * CB + g2 * 128 : I * CB + (g2 + 1) * 128],
                            ident[:],
                        )
                    nc.vector.tensor_copy(
                        oTs[g2][:, igrp * 512 : (igrp + 1) * 512], tp[:]
                    )

                # partial output projection for seq block J. split_out: two
                # column-half DMAs so the last block's drain pipelines.
                def emit_fin(J, split_out=False):
                    fs = fsp.tile([128, E], BF16 if OUT_BF16 else F32, tag="fs")
                    for half in range(2):
                        fp = fpool.tile([128, 512], F32, tag="fp")
                        for cc in range(2):
                            nc.tensor.matmul(
                                fp[:],
                                oTs[cc][:, J * 128 : (J + 1) * 128],
                                wot_sb[
                                    :, cc * E + half * 512 : cc * E + (half + 1) * 512
                                ],
                                start=(cc == 0),
                                stop=(cc == 1),
                            )
                        if half == 0:
                            nc.vector.tensor_copy(
                                fs[:, half * 512 : (half + 1) * 512], fp[:]
                            )
                        else:
                            nc.scalar.copy(
                                fs[:, half * 512 : (half + 1) * 512], fp[:]
                            )
                        if split_out:
                            nc.sync.dma_start(
                                outp[
                                    J * 128 : (J + 1) * 128,
                                    half * 512 : (half + 1) * 512,
                                ],
                                fs[:, half * 512 : (half + 1) * 512],
                            )
                    if not split_out:
                        nc.sync.dma_start(outp[J * 128 : (J + 1) * 128, :], fs[:])

                # PE pipeline: convA -> convB(h0) -> transA -> convB(h1) ->
                # finA -> convB(h2,h3) -> transB -> finB
                for _r in range(max(1, reps.get("conv", 0))):
                    for h in range(HPC):
                        emit_conv(h, 0)
                    emit_conv(0, 1)
                for _r in range(max(1, reps.get("trans", 0))):
                    emit_trans(0, 0)
                    emit_trans(1, 0)
                    emit_trans(0, 1)
                    emit_trans(1, 1)
                for _r in range(max(1, reps.get("conv", 0))):
                    emit_conv(1, 1)
                for _r in range(max(1, reps.get("fin", 0))):
                    for J in range(0, 8):
                        emit_fin(J)
                for _r in range(max(1, reps.get("conv", 0))):
                    emit_conv(2, 1)
                    emit_conv(3, 1)
                for _r in range(max(1, reps.get("trans", 0))):
                    emit_trans(0, 2)
                    emit_trans(1, 2)
                    emit_trans(0, 3)
                    emit_trans(1, 3)
                for _r in range(max(1, reps.get("fin", 0))):
                    for J in range(8, NB):
                        emit_fin(J, split_out=(J == NB - 1))

    _split_excess_waits(nc)
    return nc


class _Runner:
    """Builds the Bass module once and keeps the jitted shard_map executable."""

    def __init__(self, spec=DEFAULT_SPEC):
        import jax
        from jax.sharding import Mesh, PartitionSpec

        try:
            from jax.experimental.shard_map import shard_map
        except ImportError:
            from jax.shard_map import shard_map

        from concourse import bass2jax

        bass2jax.install_neuronx_cc_hook()
        self.jax = jax
        nc = _build_nc(spec)
        self.nc = nc

        partition_name = (
            nc.partition_id_tensor.name if nc.partition_id_tensor else None
        )
        in_names, out_names, out_avals, zero_outs = [], [], [], []
        for alloc in nc.m.functions[0].allocations:
            if not isinstance(alloc, mybir.MemoryLocationSet):
                continue
            name = alloc.memorylocations[0].name
            if alloc.kind == "ExternalInput":
                if name != partition_name:
                    in_names.append(name)
            elif alloc.kind == "ExternalOutput":
                shape = tuple(alloc.tensor_shape)
                dtype = mybir.dt.np(alloc.dtype)
                out_names.append(name)
                out_avals.append(jax.core.ShapedArray(shape, dtype))
                zero_outs.append(np.zeros(shape, dtype))
        self.in_names = in_names
        self.out_names = out_names
        self.out_shapes = [tuple(a.shape) for a in out_avals]
        self.zero_outs = zero_outs
        n_params = len(in_names)
        n_outs = len(out_names)
        all_in_names = list(in_names) + list(out_names)
        if partition_name is not None:
            all_in_names.append(partition_name)

        def _body(*args):
            operands = list(args)
            if partition_name is not None:
                operands.append(bass2jax.partition_id_tensor())
            outs = bass2jax._bass_exec_p.bind(
                *operands,
                out_avals=tuple(out_avals),
                in_names=tuple(all_in_names),
                out_names=tuple(out_names),
                lowering_input_output_aliases=(),
                sim_require_finite=True,
                sim_require_nnan=True,
                nc=nc,
            )
            return tuple(outs)

        devices = jax.devices()[:NCORES]
        assert len(devices) == NCORES, f"need {NCORES} cores, got {len(devices)}"
        self.mesh = Mesh(np.asarray(devices), ("core",))
        in_specs = (PartitionSpec("core"),) * (n_params + n_outs)
        out_specs = (PartitionSpec("core"),) * n_outs
        donate = tuple(range(n_params, n_params + n_outs))
        self.sharded = jax.jit(
            shard_map(
                _body,
                mesh=self.mesh,
                in_specs=in_specs,
                out_specs=out_specs,
                check_rep=False,
            ),
            donate_argnums=donate,
            keep_unused=True,
        )
        # Non-donating variant for benchmarking: one zeros set can be reused
        # across dispatches (kernel writes every output element).
        self.sharded_nodonate = jax.jit(
            shard_map(
                _body,
                mesh=self.mesh,
                in_specs=in_specs,
                out_specs=out_specs,
                check_rep=False,
            ),
            keep_unused=True,
        )

    def concat_inputs(self, in_maps):
        return [
            np.concatenate([np.asarray(in_maps[c][nm]) for c in range(NCORES)], axis=0)
            for nm in self.in_names
        ]

    def fresh_zeros(self):
        return [
            np.zeros((NCORES * z.shape[0], *z.shape[1:]), z.dtype)
            for z in self.zero_outs
        ]

    def run_concat(self, concat_in, zeros):
        out_arrs = self.sharded(*concat_in, *zeros)
        return out_arrs

    def run(self, in_maps):
        out_arrs = self.run_concat(self.concat_inputs(in_maps), self.fresh_zeros())
        res = []
        for c in range(NCORES):
            res.append(
                {
                    nm: np.asarray(out_arrs[i]).reshape(
                        NCORES, *self.out_shapes[i]
                    )[c]
                    for i, nm in enumerate(self.out_names)
                }
            )
        return res


_RUNNERS = {}


def _get_runner(spec=DEFAULT_SPEC):
    spec = tuple(sorted(dict(spec).items()))
    if spec not in _RUNNERS:
        _RUNNERS[spec] = _Runner(spec)
    return _RUNNERS[spec]


def _shard_inputs(x, W_A, W_V, W_O):
    x = np.asarray(x, dtype=np.float32)
    W_A = np.asarray(W_A, dtype=np.float32)
    W_V = np.asarray(W_V, dtype=np.float32)
    W_O = np.asarray(W_O, dtype=np.float32)
    xTs = [np.ascontiguousarray(x[b].T).astype(NPBF16) for b in range(B)]

    def sb_layout(wT, nk):
        # [nk*128, c] -> [128, nk*c]: partition p holds chunk-k cols at k*c
        c = wT.shape[1]
        return np.ascontiguousarray(
            wT.reshape(nk, 128, c).transpose(1, 0, 2).reshape(128, nk * c)
        ).astype(NPBF16)

    in_maps = []
    for c in range(NCORES):
        b, g = divmod(c, NCORES // B)
        r0, r1 = g * CB, (g + 1) * CB
        in_maps.append(
            {
                "xT": xTs[b],
                "wat": sb_layout(W_A[g * HPC : (g + 1) * HPC, :].T, KE),
                "wvt": sb_layout(W_V[r0:r1, :].T, KE),
                "wot": sb_layout(W_O[:, r0:r1].T, 2),
            }
        )
    return in_maps


def kernel(x, W_A, W_V, W_O, b_O):
    runner = _get_runner()
    in_maps = _shard_inputs(x, W_A, W_V, W_O)
    res = runner.run(in_maps)
    b_O = np.asarray(b_O, dtype=np.float32)
    out = np.empty((B, S, E), np.float32)
    gpb = NCORES // B
    for b in range(B):
        acc = res[b * gpb]["outp"].astype(np.float32)
        for g in range(1, gpb):
            acc = acc + res[b * gpb + g]["outp"].astype(np.float32)
        out[b] = acc + b_O
    return out


def _marginal_once(runner, dev_in, zset, k_small=4, k_big=64):
    import time

    def run_k(k):
        t0 = time.perf_counter()
        outs = None
        for _ in range(k):
            outs = runner.sharded_nodonate(*dev_in, *zset)
        for a in outs:
            a.block_until_ready()
        return time.perf_counter() - t0

    t_small = run_k(k_small)
    t_big = run_k(k_big)
    return (t_big - t_small) / (k_big - k_small) * 1e6


def measure_exec_ns(x, W_A, W_V, W_O, b_O, amp=17, pairs=11):
    """Per-execution device time: interleaved paired marginals of the normal
    kernel vs an `amp`-times-repeated body (drift-cancelling)."""
    import jax
    from jax.sharding import NamedSharding, PartitionSpec

    in_maps = _shard_inputs(x, W_A, W_V, W_O)
    setups = {}
    for factor in (1, amp):
        spec = tuple((p, factor) for p in ("z", "v", "conv", "fin", "trans"))
        runner = _get_runner(spec)
        sh = NamedSharding(runner.mesh, PartitionSpec("core"))
        dev_in = [jax.device_put(a, sh) for a in runner.concat_inputs(in_maps)]
        zset = [jax.device_put(z, sh) for z in runner.fresh_zeros()]
        for a in zset:
            a.block_until_ready()
        # warm
        _marginal_once(runner, dev_in, zset, 1, 2)
        setups[factor] = (runner, dev_in, zset)
    diffs = []
    m1s, mAs = [], []
    for _ in range(pairs):
        m1 = _marginal_once(*setups[1])
        mA = _marginal_once(*setups[amp])
        m1s.append(m1)
        mAs.append(mA)
        diffs.append((mA - m1) / (amp - 1))
    diffs.sort()
    med = diffs[len(diffs) // 2]
    return {
        "m1_us": [round(v) for v in m1s],
        f"m{amp}_us": [round(v) for v in mAs],
        "diffs_us": [round(v, 1) for v in sorted(diffs)],
        "per_exec_ns": int(med * 1e3),
    }


# revision 34
# speedup vs baseline: 4.6488x; 1.4917x over previous
"""CATAttention Trainium2 kernel (v2 — bf16, DMA/pipeline restructure).

Math: out[b,i,h,:] = sum_{j<=i} softmax_s(x@W_A^T)[b,i-j,h] * v[b,j,h,:]
i.e. a causal convolution along the sequence with a per-(b,h) data-dependent
kernel z. The [B,H,S,S] "roll" matrix is block-Toeplitz: its 128x128 blocks
depend only on the block lag L = I-J, so only 16 distinct blocks per head are
ever materialized. They are built UNNORMALIZED (from ez = exp(logits)) by a
single negative-stride sliding-window DMA per head from a zero-padded copy of
ez in DRAM; the 1/sum(ez) softmax normalizer is folded into the PSUM->SBUF
copy of the conv output (per-head scalar).

Sharding (8 cores): core c -> batch b = c//4, head group g = c%4 (4 heads).
Each core computes z, v = x@W_V^T (its 256 channels), the causal Toeplitz
matmul, and a partial output projection against its 256 columns of W_O.
Host gathers: out[b] = sum of the 4 partials + b_O.

All data bf16 (f32 PSUM accumulation). Input DMAs are 4 fat quarter loads
with the weights interleaved so the z/v matmuls start at ~3us, not ~26us.
"""

import numpy as np

import concourse.bass as bass
import concourse.mybir as mybir
import concourse.tile as tile
from concourse import masks
from concourse.ap import AP

F32 = mybir.dt.float32
BF16 = mybir.dt.bfloat16
NPBF16 = mybir.dt.np(BF16)
OUT_BF16 = True

B, S, E, H, D = 2, 2048, 1024, 16, 64
SCALING = D ** -0.5
NCORES = 8
HPC = 4            # heads per core
CB = HPC * D       # 256 channels per core
NB = S // 128      # 16 seq blocks
KE = E // 128      # 8 contraction chunks
ZW = S + 128       # zpad row: 128 zeros + 2048 ez values


def _split_excess_waits(nc, max_waits=1):
    """The walrus in this container rejects >2 sync waits per instruction.
    Hoist excess waits onto standalone EventSemaphore insts on the same engine."""
    ctr = 0
    for fn in nc.m.functions:
        for bb in fn.blocks:
            out = []
            changed = False
            for inst in list(bb.instructions):
                si = inst.sync_info
                if si is not None and si.on_wait and len(si.on_wait) > max_waits:
                    extra = list(si.on_wait[:-max_waits])
                    keep = list(si.on_wait[-max_waits:])
                    for w in extra:
                        ctr += 1
                        ev = mybir.InstEventSemaphore(
                            name=f"I-waitsplit-{ctr}", ins=[], outs=[]
                        )
                        ev.engine = inst.engine
                        ev.sync_info = mybir.SyncInfo(on_wait=[w], on_update=[])
                        out.append(ev)
                    si.on_wait = keep
                    changed = True
                out.append(inst)
            if changed:
                bb.instructions = out
    return ctr


DEFAULT_SPEC = (("conv", 1), ("fin", 1), ("trans", 1), ("v", 1), ("z", 1))


def _build_nc(spec=DEFAULT_SPEC):
    reps = dict(spec)
    nc = bass.Bass()
    xT = nc.dram_tensor("xT", [E, S], BF16, kind="ExternalInput")
    # weights arrive host-pre-arranged in the exact SBUF layout (contiguous DMA)
    wat = nc.dram_tensor("wat", [128, KE * HPC], BF16, kind="ExternalInput")
    wvt = nc.dram_tensor("wvt", [128, KE * CB], BF16, kind="ExternalInput")
    wot = nc.dram_tensor("wot", [128, 2 * E], BF16, kind="ExternalInput")
    outp = nc.dram_tensor(
        "outp", [S, E], BF16 if OUT_BF16 else F32, kind="ExternalOutput"
    )
    zpad = nc.dram_tensor("zpad", [HPC, ZW], BF16)
    rzd = nc.dram_tensor("rzd", [1, HPC], F32)

    with tile.TileContext(nc) as tc:
        with (
            tc.tile_pool(name="per", bufs=1) as per,
            tc.tile_pool(name="fs", bufs=4) as fsp,
            tc.tile_pool(name="stp", bufs=2) as stp,
        ):
            ident = per.tile([128, 128], BF16, tag="ident")
            masks.make_identity(nc, ident[:])

            # --- input loads: weights interleaved with 4 fat x-quarter DMAs
            wat_sb = per.tile([128, KE * HPC], BF16, tag="wat")
            wvt_sb = per.tile([128, KE * CB], BF16, tag="wvt")
            wot_sb = per.tile([128, 2 * E], BF16, tag="wot")
            xT_sb = per.tile([128, KE * S], BF16, tag="xT")
            x3 = xT_sb[:].rearrange("p (k c) -> p k c", k=KE)

            def load_xq(n, k0=0, k1=KE):
                # one DMA: k-chunks [k0,k1) columns [n*512, (n+1)*512)
                nc.sync.dma_start(
                    x3[:, k0:k1, n * 512 : (n + 1) * 512],
                    AP(
                        xT,
                        k0 * 128 * S + n * 512,
                        [[S, 128], [128 * S, k1 - k0], [1, 512]],
                    ),
                )

            nc.sync.dma_start(wat_sb[:], wat[:])
            load_xq(0, 0, 4)
            load_xq(0, 4, KE)
            nc.sync.dma_start(wvt_sb[:], wvt[:])
            for n in range(1, 4):
                load_xq(n)
            nc.sync.dma_start(wot_sb[:], wot[:])

            # zero tail of zpad (t=0, Pool queue); exp act-table preload (Act)
            zero128 = per.tile([HPC, 128], BF16, tag="zero")
            nc.vector.memset(zero128[:], 0.0)
            nc.gpsimd.dma_start(
                AP(zpad, S, [[ZW, HPC], [1, 128]]), zero128[:]
            )
            dumm = per.tile([1, 1], F32, tag="dumm")
            nc.scalar.activation(
                dumm[:], zero128[0:1, 0:1], mybir.ActivationFunctionType.Exp
            )

            ez = per.tile([HPC, S], BF16, tag="ez")
            ezR = per.tile([HPC, S], BF16, tag="ezR")
            zsum2 = per.tile([HPC, 2], F32, tag="zsum2")
            zsum = per.tile([HPC, 1], F32, tag="zsum")
            rz128 = per.tile([128, HPC], F32, tag="rz128")
            rzr = per.tile([128, HPC], F32, tag="rzr")

            v_sb = per.tile([128, NB * CB], BF16, tag="v")
            o_sb = per.tile([128, NB * CB], BF16, tag="o")
            oTs = [
                per.tile([128, S], BF16, tag=f"oT{g2}", name=f"oT{g2}")
                for g2 in range(2)
            ]
            a_sb = per.tile([128, HPC * S], BF16, tag="a")

            with (
                tc.tile_pool(name="zp", bufs=1, space="PSUM") as zpool,
                tc.tile_pool(name="vp", bufs=3, space="PSUM") as vpool,
            ):
                zp = zpool.tile([HPC, S], F32, tag="zp")

                def emit_z_matmuls(n):
                    for k in range(KE):
                        nc.tensor.matmul(
                            zp[:, n * 512 : (n + 1) * 512],
                            wat_sb[:, k * HPC : (k + 1) * HPC],
                            x3[:, k, n * 512 : (n + 1) * 512],
                            start=(k == 0),
                            stop=(k == KE - 1),
                            skip_group_check=True,
                        )

                def emit_v_block(J):
                    vp = vpool.tile([128, CB], F32, tag="vp")
                    for k in range(KE):
                        nc.tensor.matmul(
                            vp[:],
                            x3[:, k, J * 128 : (J + 1) * 128],
                            wvt_sb[:, k * CB : (k + 1) * CB],
                            start=(k == 0),
                            stop=(k == KE - 1),
                        )
                    nc.vector.tensor_copy(v_sb[:, J * CB : (J + 1) * CB], vp[:])

                stages = []

                def emit_exp_half(hf):
                    # exp reads the z PSUM directly; sum rides along in accum.
                    # zpad rows hold ez REVERSED: zpad[h,m] = ez[h, S-1-m],
                    # zero tail at [S, ZW) (written at t=0); data writes go
                    # on the (idle) gpsimd SWDGE queue.
                    c0, c1 = hf * 1024, (hf + 1) * 1024
                    nc.scalar.activation(
                        ez[:, c0:c1], zp[:, c0:c1],
                        mybir.ActivationFunctionType.Exp,
                        scale=SCALING, accum_out=zsum2[:, hf : hf + 1],
                    )
                    r0 = S - c1  # reversed position of this half
                    nc.vector.tensor_copy(
                        ezR[:, r0 : r0 + 1024], ez[:, c0:c1][:, ::-1]
                    )
                    # SP queue is idle mid-kernel; keeps gpsimd free for copies
                    nc.sync.dma_start(
                        AP(zpad, r0, [[ZW, HPC], [1, 1024]]),
                        ezR[:, r0 : r0 + 1024],
                    )

                def emit_stage():
                    nc.vector.reduce_sum(
                        zsum[:], zsum2[:], axis=mybir.AxisListType.X
                    )
                    # zsum broadcast to all partitions via DRAM bounce
                    # ([4,1] partitions -> 4 consecutive floats -> [128,4])
                    nc.sync.dma_start(
                        AP(rzd, 0, [[1, HPC], [1, 1]]), zsum[:]
                    )
                    nc.sync.dma_start(
                        rz128[:], AP(rzd, 0, [[0, 128], [1, HPC]])
                    )
                    # stage_h[j, m] = zpad[h, j + m]  (sliding window; zpad
                    # holds ez reversed, zero tail) -> the whole-row reversed
                    # DVE copy below materializes all 16 lag blocks at once:
                    # a_sb[j, h*S + q] = stage_h[j, S-1-q] = ez_ext[q - j]
                    stages.clear()
                    for h in range(HPC):
                        st = stp.tile([128, S], BF16, tag="stage", name=f"st{h}")
                        nc.scalar.dma_start(
                            st[:], AP(zpad, h * ZW, [[1, 128], [1, S]])
                        )
                        stages.append(st)

                def emit_rev(h):
                    nc.vector.tensor_copy(
                        a_sb[:, h * S : (h + 1) * S], stages[h][:, ::-1]
                    )

                # PE order: each z quarter as soon as its x quarter lands,
                # v blocks filling the slack
                def emit_v_range(j0, j1):
                    for _r in range(max(1, reps.get("v", 0))):
                        for J in range(j0, j1):
                            emit_v_block(J)

                for _r in range(reps.get("z", 0)):
                    emit_z_matmuls(0)
                emit_v_range(0, 3)
                for _r in range(reps.get("z", 0)):
                    emit_z_matmuls(1)
                    emit_exp_half(0)
                emit_v_range(3, 5)
                for _r in range(reps.get("z", 0)):
                    emit_z_matmuls(2)
                emit_v_range(5, 7)
                for _r in range(reps.get("z", 0)):
                    emit_z_matmuls(3)
                    emit_exp_half(1)
                    emit_stage()
                emit_v_range(7, 14)
                for _r in range(reps.get("z", 0)):
                    for h in range(HPC):
                        emit_rev(h)
                emit_v_range(14, NB)

            v3 = v_sb[:].rearrange("p (j c) -> p j c", c=CB)
            o3 = o_sb[:].rearrange("p (i c) -> p i c", c=CB)
            with (
                tc.tile_pool(name="op", bufs=3, space="PSUM") as opool,
                tc.tile_pool(name="tp", bufs=2, space="PSUM") as tpool,
                tc.tile_pool(name="fp", bufs=3, space="PSUM") as fpool,
            ):
                # softmax normalizer, needed first by the conv copy-out
                nc.vector.reciprocal(rzr[:], rz128[:])

                # causal Toeplitz matmul, halved: half 0 = out blocks I 0..7,
                # half 1 = I 8..15. Per (half, head): one PSUM bank.
                def emit_conv(h, half):
                    op = opool.tile([128, 512], F32, tag="op")
                    if half == 0:
                        for L in range(8):
                            aT = a_sb[:, (h * 16 + L) * 128 : (h * 16 + L + 1) * 128]
                            nc.tensor.matmul(
                                op[:, L * 64 : 512],
                                aT,
                                v3[:, 0 : 8 - L, h * 64 : (h + 1) * 64],
                                start=(L == 0),
                                stop=(L == 7),
                                skip_group_check=True,
                            )
                        dst = o3[:, 0:8, h * 64 : (h + 1) * 64]
                    else:
                        for L in range(16):
                            aT = a_sb[:, (h * 16 + L) * 128 : (h * 16 + L + 1) * 128]
                            j0 = max(0, 8 - L)
                            nc.tensor.matmul(
                                op[:, (max(8, L) - 8) * 64 : 512],
                                aT,
                                v3[:, j0 : 16 - L, h * 64 : (h + 1) * 64],
                                start=(L == 0),
                                stop=(L == 15),
                                skip_group_check=True,
                            )
                        dst = o3[:, 8:NB, h * 64 : (h + 1) * 64]
                    # fold softmax 1/sum into the copy-out; spread the copies
                    # over DVE (heads 0,1) and Act (heads 2,3)
                    src = op[:].rearrange("p (i c) -> p i c", c=64)
                    if h < 2:
                        nc.vector.tensor_scalar_mul(dst, src, rzr[:, h : h + 1])
                    else:
                        nc.scalar.mul(dst, src, rzr[:, h : h + 1])

                # transpose o -> oT for out blocks I in [i0, i0+4), group g2
                def emit_trans(g2, igrp):
                    tp = tpool.tile([128, 512], BF16, tag="tp")
                    for i in range(4):
                        I = igrp * 4 + i
                        nc.tensor.transpose(
                            tp[:, i * 128 : (i + 1) * 128],
                            o_sb[:, I * CB + g2 * 128 : I * CB + (g2 + 1) * 128],
                            ident[:],
                        )
                    nc.scalar.copy(
                        oTs[g2][:, igrp * 512 : (igrp + 1) * 512], tp[:]
                    )

                # partial output projection for seq block J. split_out: two
                # column-half DMAs so the last block's drain pipelines.
                def emit_fin(J, split_out=False):
                    fs = fsp.tile([128, E], BF16 if OUT_BF16 else F32, tag="fs")
                    for half in range(2):
                        fp = fpool.tile([128, 512], F32, tag="fp")
                        for cc in range(2):
                            nc.tensor.matmul(
                                fp[:],
                                oTs[cc][:, J * 128 : (J + 1) * 128],
                                wot_sb[
                                    :, cc * E + half * 512 : cc * E + (half + 1) * 512
                                ],
                                start=(cc == 0),
                                stop=(cc == 1),
                            )
                        if half == 0:
                            nc.vector.tensor_copy(
                                fs[:, half * 512 : (half + 1) * 512], fp[:]
                            )
                        else:
                            nc.scalar.copy(
                                fs[:, half * 512 : (half + 1) * 512], fp[:]
                            )
                        if split_out:
                            nc.sync.dma_start(
                                outp[
                                    J * 128 : (J + 1) * 128,
                                    half * 512 : (half + 1) * 512,
                                ],
                                fs[:, half * 512 : (half + 1) * 512],
                            )
                    if not split_out:
                        nc.sync.dma_start(outp[J * 128 : (J + 1) * 128, :], fs[:])

                # PE pipeline: convA -> convB(h0) -> transA -> convB(h1) ->
                # finA -> convB(h2,h3) -> transB -> finB
                for _r in range(max(1, reps.get("conv", 0))):
                    for h in range(HPC):
                        emit_conv(h, 0)
                    emit_conv(0, 1)
                for _r in range(max(1, reps.get("trans", 0))):
                    emit_trans(0, 0)
                    emit_trans(1, 0)
                    emit_trans(0, 1)
                    emit_trans(1, 1)
                for _r in range(max(1, reps.get("conv", 0))):
                    emit_conv(1, 1)
                for _r in range(max(1, reps.get("fin", 0))):
                    for J in range(0, 8):
                        emit_fin(J)
                for _r in range(max(1, reps.get("conv", 0))):
                    emit_conv(2, 1)
                    emit_conv(3, 1)
                for _r in range(max(1, reps.get("trans", 0))):
                    emit_trans(0, 2)
                    emit_trans(1, 2)
                    emit_trans(0, 3)
                    emit_trans(1, 3)
                for _r in range(max(1, reps.get("fin", 0))):
                    for J in range(8, NB):
                        emit_fin(J, split_out=(J == NB - 1))

    _split_excess_waits(nc)
    return nc


class _Runner:
    """Builds the Bass module once and keeps the jitted shard_map executable."""

    def __init__(self, spec=DEFAULT_SPEC):
        import jax
        from jax.sharding import Mesh, PartitionSpec

        try:
            from jax.experimental.shard_map import shard_map
        except ImportError:
            from jax.shard_map import shard_map

        from concourse import bass2jax

        bass2jax.install_neuronx_cc_hook()
        self.jax = jax
        nc = _build_nc(spec)
        self.nc = nc

        partition_name = (
            nc.partition_id_tensor.name if nc.partition_id_tensor else None
        )
        in_names, out_names, out_avals, zero_outs = [], [], [], []
        for alloc in nc.m.functions[0].allocations:
            if not isinstance(alloc, mybir.MemoryLocationSet):
                continue
            name = alloc.memorylocations[0].name
            if alloc.kind == "ExternalInput":
                if name != partition_name:
                    in_names.append(name)
            elif alloc.kind == "ExternalOutput":
                shape = tuple(alloc.tensor_shape)
                dtype = mybir.dt.np(alloc.dtype)
                out_names.append(name)
                out_avals.append(jax.core.ShapedArray(shape, dtype))
                zero_outs.append(np.zeros(shape, dtype))
        self.in_names = in_names
        self.out_names = out_names
        self.out_shapes = [tuple(a.shape) for a in out_avals]
        self.zero_outs = zero_outs
        n_params = len(in_names)
        n_outs = len(out_names)
        all_in_names = list(in_names) + list(out_names)
        if partition_name is not None:
            all_in_names.append(partition_name)

        def _body(*args):
            operands = list(args)
            if partition_name is not None:
                operands.append(bass2jax.partition_id_tensor())
            outs = bass2jax._bass_exec_p.bind(
                *operands,
                out_avals=tuple(out_avals),
                in_names=tuple(all_in_names),
                out_names=tuple(out_names),
                lowering_input_output_aliases=(),
                sim_require_finite=True,
                sim_require_nnan=True,
                nc=nc,
            )
            return tuple(outs)

        devices = jax.devices()[:NCORES]
        assert len(devices) == NCORES, f"need {NCORES} cores, got {len(devices)}"
        self.mesh = Mesh(np.asarray(devices), ("core",))
        in_specs = (PartitionSpec("core"),) * (n_params + n_outs)
        out_specs = (PartitionSpec("core"),) * n_outs
        donate = tuple(range(n_params, n_params + n_outs))
        self.sharded = jax.jit(
            shard_map(
                _body,
                mesh=self.mesh,
                in_specs=in_specs,
                out_specs=out_specs,
                check_rep=False,
            ),
            donate_argnums=donate,
            keep_unused=True,
        )
        # Non-donating variant for benchmarking: one zeros set can be reused
        # across dispatches (kernel writes every output element).
        self.sharded_nodonate = jax.jit(
            shard_map(
                _body,
                mesh=self.mesh,
                in_specs=in_specs,
                out_specs=out_specs,
                check_rep=False,
            ),
            keep_unused=True,
        )

    def concat_inputs(self, in_maps):
        return [
            np.concatenate([np.asarray(in_maps[c][nm]) for c in range(NCORES)], axis=0)
            for nm in self.in_names
        ]

    def fresh_zeros(self):
        return [
            np.zeros((NCORES * z.shape[0], *z.shape[1:]), z.dtype)
            for z in self.zero_outs
        ]

    def run_concat(self, concat_in, zeros):
        out_arrs = self.sharded(*concat_in, *zeros)
        return out_arrs

    def run(self, in_maps):
        out_arrs = self.run_concat(self.concat_inputs(in_maps), self.fresh_zeros())
        res = []
        for c in range(NCORES):
            res.append(
                {
                    nm: np.asarray(out_arrs[i]).reshape(
                        NCORES, *self.out_shapes[i]
                    )[c]
                    for i, nm in enumerate(self.out_names)
                }
            )
        return res


_RUNNERS = {}


def _get_runner(spec=DEFAULT_SPEC):
    spec = tuple(sorted(dict(spec).items()))
    if spec not in _RUNNERS:
        _RUNNERS[spec] = _Runner(spec)
    return _RUNNERS[spec]


def _shard_inputs(x, W_A, W_V, W_O):
    x = np.asarray(x, dtype=np.float32)
    W_A = np.asarray(W_A, dtype=np.float32)
    W_V = np.asarray(W_V, dtype=np.float32)
    W_O = np.asarray(W_O, dtype=np.float32)
    xTs = [np.ascontiguousarray(x[b].T).astype(NPBF16) for b in range(B)]

    def sb_layout(wT, nk):
        # [nk*128, c] -> [128, nk*c]: partition p holds chunk-k cols at k*c
        c = wT.shape[1]
        return np.ascontiguousarray(
            wT.reshape(nk, 128, c).transpose(1, 0, 2).reshape(128, nk * c)
        ).astype(NPBF16)

    in_maps = []
    for c in range(NCORES):
        b, g = divmod(c, NCORES // B)
        r0, r1 = g * CB, (g + 1) * CB
        in_maps.append(
            {
                "xT": xTs[b],
                "wat": sb_layout(W_A[g * HPC : (g + 1) * HPC, :].T, KE),
                "wvt": sb_layout(W_V[r0:r1, :].T, KE),
                "wot": sb_layout(W_O[:, r0:r1].T, 2),
            }
        )
    return in_maps


def kernel(x, W_A, W_V, W_O, b_O):
    runner = _get_runner()
    in_maps = _shard_inputs(x, W_A, W_V, W_O)
    res = runner.run(in_maps)
    b_O = np.asarray(b_O, dtype=np.float32)
    out = np.empty((B, S, E), np.float32)
    gpb = NCORES // B
    for b in range(B):
        acc = res[b * gpb]["outp"].astype(np.float32)
        for g in range(1, gpb):
            acc = acc + res[b * gpb + g]["outp"].astype(np.float32)
        out[b] = acc + b_O
    return out


def _marginal_once(runner, dev_in, zset, k_small=8, k_big=96):
    import time

    def run_k(k):
        t0 = time.perf_counter()
        outs = None
        for _ in range(k):
            outs = runner.sharded_nodonate(*dev_in, *zset)
        for a in outs:
            a.block_until_ready()
        return time.perf_counter() - t0

    t_small = run_k(k_small)
    t_big = run_k(k_big)
    return (t_big - t_small) / (k_big - k_small) * 1e6


def measure_exec_ns(x, W_A, W_V, W_O, b_O, amp=17, pairs=13):
    """Per-execution device time: interleaved paired marginals of the normal
    kernel vs an `amp`-times-repeated body (drift-cancelling)."""
    import jax
    from jax.sharding import NamedSharding, PartitionSpec

    in_maps = _shard_inputs(x, W_A, W_V, W_O)
    setups = {}
    for factor in (1, amp):
        spec = tuple((p, factor) for p in ("z", "v", "conv", "fin", "trans"))
        runner = _get_runner(spec)
        sh = NamedSharding(runner.mesh, PartitionSpec("core"))
        dev_in = [jax.device_put(a, sh) for a in runner.concat_inputs(in_maps)]
        zset = [jax.device_put(z, sh) for z in runner.fresh_zeros()]
        for a in zset:
            a.block_until_ready()
        # warm
        _marginal_once(runner, dev_in, zset, 1, 2)
        setups[factor] = (runner, dev_in, zset)
    diffs = []
    m1s, mAs = [], []
    for _ in range(pairs):
        m1 = _marginal_once(*setups[1])
        mA = _marginal_once(*setups[amp])
        m1s.append(m1)
        mAs.append(mA)
        diffs.append((mA - m1) / (amp - 1))
    diffs.sort()
    med = diffs[len(diffs) // 2]
    return {
        "m1_us": [round(v) for v in m1s],
        f"m{amp}_us": [round(v) for v in mAs],
        "diffs_us": [round(v, 1) for v in sorted(diffs)],
        "per_exec_ns": int(med * 1e3),
    }
